# revision 17
# baseline (speedup 1.0000x reference)
"""Trainium2 Bass kernel for Mistral sliding-window attention (B=1, S=4096,
HID=1024, H=8 q-heads, KVH=2 kv-heads, D=128, WINDOW=2048).

Strategy: shard the 4096-token sequence across 8 NeuronCores (512 queries per
core). Each core recomputes the K/V projections for its 2560-row key window
(own 512 rows + previous 2048), applies RoPE, computes sliding-window causal
attention for all 8 heads in the [keys, queries] orientation (scores^T), and
runs the full o_proj for its 512 rows. No collectives are needed; the host
concatenates the per-core 512-row output blocks.

Softmax is computed without max-subtraction (scores are O(1) here): P=exp(s)
via ScalarE with the denominator obtained as an extra ones-row matmul on the
TensorE, corrected on device for zero-padded (out-of-range) keys via a
host-provided count, since padded keys contribute exp(0)=1 to the sum but 0
to P@V (their V rows are 0).
"""

import sys
import numpy as np
from contextlib import ExitStack

if "/opt/trn_rl_repo" not in sys.path:
    sys.path.insert(0, "/opt/trn_rl_repo")

# ---------------------------------------------------------------- constants
FULL_CFG = dict(
    R=512,        # queries per core
    W=2048,       # sliding window
    HID=1024,     # hidden size
    H=8,          # query heads
    KVH=2,        # kv heads
    D=128,        # head dim
    THETA=10000.0,
    GRP=2,        # score chunks per exp batch
    NCORES=8,
)


def _derived(cfg):
    R, W, HID = cfg["R"], cfg["W"], cfg["HID"]
    KVW = W + R
    HC = HID // 128
    NKC = KVW // 128
    assert W >= R and R <= 512 and HID % 128 == 0 and KVW % 128 == 0
    return KVW, HC, NKC


def build_program(cfg):
    import concourse.bass as bass
    import concourse.tile as tile
    from concourse import bacc, mybir

    f32, f16 = mybir.dt.float32, mybir.dt.float16
    ts = bass.ts
    R, W, HID, H, KVH, D, GRP = (cfg["R"], cfg["W"], cfg["HID"], cfg["H"],
                                 cfg["KVH"], cfg["D"], cfg["GRP"])
    KVW, HC, NKC = _derived(cfg)
    GROUPS = H // KVH
    LO = R // 128          # lower-edge chunks: kc in [0, LO)
    HIE = W // 128         # upper-edge chunks: kc in [HIE, NKC)

    nc = bacc.Bacc("TRN2", target_bir_lowering=False, debug=False)

    def din(name, shape, dt):
        return nc.dram_tensor(name, shape, dt, kind="ExternalInput").ap()

    xT = din("xT", [128 * HC * KVW], f16)       # block-major packed
    wqT = din("wqT", [128, HC * H * D], f16)    # partition-major packed
    wkT = din("wkT", [128, HC * KVH * D], f16)
    wvT = din("wvT", [128, HC * KVH * D], f16)
    woT = din("woT", [128, H * HID], f16)
    cosK = din("cosK", [128, KVW], f16)
    sinK = din("sinK", [128, KVW], f16)
    cosQ = din("cosQ", [128, R], f16)
    sinQ = din("sinQ", [128, R], f16)
    npadQ = din("npadQ", [1, R], f32)
    outT = nc.dram_tensor("outT", [HC, 128, R], f32, kind="ExternalOutput").ap()

    with tile.TileContext(nc) as tc, ExitStack() as ctx:
        const = ctx.enter_context(tc.tile_pool(name="const", bufs=1))
        kvp = ctx.enter_context(tc.tile_pool(name="kvp", bufs=1))
        work = ctx.enter_context(tc.tile_pool(name="work", bufs=2))
        dramp = ctx.enter_context(tc.tile_pool(name="dramp", bufs=2, space="DRAM"))
        psG = ctx.enter_context(tc.tile_pool(name="psG", bufs=2, space="PSUM"))
        psM = ctx.enter_context(tc.tile_pool(name="psM", bufs=1, space="PSUM"))
        psC = ctx.enter_context(tc.tile_pool(name="psC", bufs=2, space="PSUM"))
        psD = ctx.enter_context(tc.tile_pool(name="psD", bufs=1, space="PSUM"))

        # ---------------- input loads
        wk_sb = const.tile([128, HC, KVH * D], f16)
        wv_sb = const.tile([128, HC, KVH * D], f16)
        nc.sync.dma_start(out=wk_sb, in_=wkT)
        nc.sync.dma_start(out=wv_sb, in_=wvT)
        cosK_sb = const.tile([128, KVW], f16)
        sinK_sb = const.tile([128, KVW], f16)
        cosQ_sb = const.tile([128, R], f16)
        sinQ_sb = const.tile([128, R], f16)
        npad_sb = const.tile([1, R], f32)
        xT_sb = const.tile([128, HC, KVW], f16)
        xblocks = []
        xoff = 0
        for b0, bw in ((o, min(512, KVW - o)) for o in range(0, KVW, 512)):
            xblocks.append((b0, bw, xoff))
            xoff += 128 * HC * bw

        def emit_xt_block(i):
            b0, bw, off = xblocks[i]
            src_ap = bass.AP(tensor=xT.tensor, offset=off,
                             ap=[[HC * bw, 128], [bw, HC], [1, bw]])
            nc.sync.dma_start(out=xT_sb[:, :, b0:b0 + bw], in_=src_ap)

        emit_xt_block(0)
        for dst, src in ((cosK_sb, cosK), (sinK_sb, sinK), (cosQ_sb, cosQ),
                         (sinQ_sb, sinQ), (npad_sb, npadQ)):
            nc.sync.dma_start(out=dst, in_=src)
        for i in range(1, len(xblocks)):
            emit_xt_block(i)
        wq_sb = const.tile([128, HC, H * D], f16)
        wo_sb = const.tile([128, H, HID], f16)

        def emit_late_loads():
            gate = gate_box[0]
            di = nc.sync.dma_start(out=wq_sb, in_=wqT)
            tile.add_dep_helper(di.ins, gate.ins, sync=True,
                                reason="delay wq load")
            di = nc.sync.dma_start(out=wo_sb, in_=woT)
            tile.add_dep_helper(di.ins, gate.ins, sync=True,
                                reason="delay wo load")
        ones_sb = const.tile([128, 1], f16)
        nc.vector.memset(ones_sb, 1.0)
        ones_row = const.tile([1, 128], f32)
        nc.vector.memset(ones_row, 1.0)

        # ---------------- additive masks for edge chunks (compile-time)
        # scores^T chunk kc holds keys jl = 128*kc + kp (kp = partition) vs
        # queries i (free). valid iff i < jl <= i + W.
        masks = {}
        for kc in list(range(LO)) + list(range(HIE, NKC)):
            m = const.tile([128, R], f16, name=f"mask{kc}")
            nc.gpsimd.memset(m, 1.0)
            if kc < LO:
                # valid iff 128*kc + kp - 1 - i >= 0
                nc.gpsimd.affine_select(
                    out=m, in_=m, compare_op=mybir.AluOpType.is_ge, fill=0.0,
                    base=128 * kc - 1, pattern=[[-1, R]], channel_multiplier=1)
            else:
                # valid iff i - kp + (W - 128*kc) >= 0
                nc.gpsimd.affine_select(
                    out=m, in_=m, compare_op=mybir.AluOpType.is_ge, fill=0.0,
                    base=W - 128 * kc, pattern=[[1, R]], channel_multiplier=-1)
            masks[kc] = m

        # ---------------- RoPE helper (operates on [128, width] psum -> sbuf)
        def rope(dst, src_ps, cos_ap, sin_ap, width):
            sb = work.tile([128, width], f32, tag="ropesrc")
            nc.vector.tensor_copy(out=sb, in_=src_ps)
            tmp = work.tile([128, width], f32, tag="rtmp")
            nc.vector.tensor_copy(out=tmp[0:64, :], in_=sb[64:128, :])
            nc.vector.tensor_copy(out=tmp[64:128, :], in_=sb[0:64, :])
            ta = work.tile([128, width], f32, tag="ra")
            nc.vector.tensor_mul(ta, sb, cos_ap)
            tb = work.tile([128, width], f32, tag="rb2")
            nc.vector.tensor_mul(tb, tmp, sin_ap)
            return nc.vector.tensor_add(dst, ta, tb)

        def blocks(total, step):
            out = []
            o = 0
            while o < total:
                out.append((o, min(step, total - o)))
                o += step
            return out

        # ---------------- K/V projections + RoPE (per kv head)
        kT_sb = [kvp.tile([128, KVW], f16, name=f"kT{g}") for g in range(KVH)]
        v_sb = [kvp.tile([128, NKC, 128], f16, name=f"v{g}") for g in range(KVH)]

        gate_box = []

        def emit_kv(g):
            for b0, bw in blocks(KVW, 512):
                kps = psM.tile([128, 512], f32, tag="mm")
                for c in range(HC):
                    nc.tensor.matmul(kps[:, :bw],
                                     lhsT=wk_sb[:, c, g * D:(g + 1) * D],
                                     rhs=xT_sb[:, c, b0:b0 + bw],
                                     start=(c == 0), stop=(c == HC - 1))
                rinst = rope(kT_sb[g][:, b0:b0 + bw], kps[:, :bw],
                             cosK_sb[:, b0:b0 + bw], sinK_sb[:, b0:b0 + bw], bw)
                if not gate_box:
                    gate_box.append(rinst)
                vps = psD.tile([128, 512], f32, tag="den")
                for c in range(HC):
                    nc.tensor.matmul(vps[:, :bw],
                                     lhsT=wv_sb[:, c, g * D:(g + 1) * D],
                                     rhs=xT_sb[:, c, b0:b0 + bw],
                                     start=(c == 0), stop=(c == HC - 1))
                vT_t = work.tile([128, 512], f16, tag="vT")
                nc.vector.tensor_copy(out=vT_t[:, :bw], in_=vps[:, :bw])
                c0 = b0 // 128
                nc.sync.dma_start_transpose(
                    out=v_sb[g][:, c0:c0 + bw // 128, :], in_=vT_t[:, :bw])

        # ---------------- attention per query head
        ctxn = [kvp.tile([128, R], f16, name=f"ctxn{h}") for h in range(H)]
        chunk_groups = [list(range(s, min(s + GRP, NKC)))
                        for s in range(0, NKC, GRP)]

        qTs = {}

        def emit_q(h):
            qps = psM.tile([128, 512], f32, tag="mm")
            for c in range(HC):
                nc.tensor.matmul(qps[:, :R], lhsT=wq_sb[:, c, h * D:(h + 1) * D],
                                 rhs=xT_sb[:, c, W:KVW],
                                 start=(c == 0), stop=(c == HC - 1))
            qT = work.tile([128, R], f16, tag="qT")
            rope(qT, qps[:, :R], cosQ_sb, sinQ_sb, R)
            qTs[h] = qT

        def emit_attn(h):
            g = h // GROUPS
            qT = qTs.pop(h)

            ctx_ps = psC.tile([128, R], f32, tag="ctx")
            den_ps = psD.tile([1, R], f32, tag="den")
            for gi, grp in enumerate(chunk_groups):
                if gi == len(chunk_groups) - 1 and h + 1 < H:
                    emit_q(h + 1)
                gw = len(grp) * R
                scps = psG.tile([128, GRP * R], f32, tag="sc")
                for j, kc in enumerate(grp):
                    nc.tensor.matmul(scps[:, ts(j, R)],
                                     lhsT=kT_sb[g][:, ts(kc, 128)], rhs=qT,
                                     start=True, stop=True)
                P_sb = work.tile([128, GRP * R], f16, tag="P", bufs=3)
                nc.scalar.activation(out=P_sb[:, :gw], in_=scps[:, :gw],
                                     func=mybir.ActivationFunctionType.Exp)
                for j, kc in enumerate(grp):
                    if kc in masks:
                        nc.vector.tensor_mul(P_sb[:, ts(j, R)],
                                             P_sb[:, ts(j, R)], masks[kc])
                for j, kc in enumerate(grp):
                    nc.tensor.matmul(ctx_ps, lhsT=v_sb[g][:, kc, :],
                                     rhs=P_sb[:, ts(j, R)],
                                     start=(kc == 0), stop=(kc == NKC - 1))
                    nc.tensor.matmul(den_ps, lhsT=ones_sb,
                                     rhs=P_sb[:, ts(j, R)],
                                     start=(kc == 0), stop=(kc == NKC - 1))

            drow = work.tile([1, R], f32, tag="drow")
            nc.vector.tensor_sub(drow, den_ps, npad_sb)
            rrow = work.tile([1, R], f32, tag="rrow")
            nc.vector.reciprocal_approx_fast(out=rrow, in_=drow)
            rbc = work.tile([128, R], f32, tag="rbc")
            if h == H - 1:
                bc_ps = psM.tile([128, 512], f32, tag="mm")
                nc.tensor.matmul(bc_ps[:, :R], lhsT=ones_row, rhs=rrow,
                                 start=True, stop=True)
                nc.vector.tensor_copy(out=rbc, in_=bc_ps[:, :R])
            else:
                rdram = dramp.tile([1, R], f32, tag="rdram")
                nc.sync.dma_start(out=rdram, in_=rrow)
                rbc_src = bass.AP(tensor=rdram.tensor, offset=rdram.offset,
                                  ap=[[0, 128]] + list(rdram.ap[-1:]))
                nc.sync.dma_start(out=rbc, in_=rbc_src)
            nc.vector.tensor_mul(ctxn[h], ctx_ps, rbc)

        emit_kv(0)
        emit_late_loads()
        for g in range(1, KVH):
            emit_kv(g)
        emit_q(0)
        for h in range(H):
            emit_attn(h)

        # ---------------- o_proj
        for ot in range(HC):
            ops = psC.tile([128, R], f32, tag="ctx")
            for h in range(H):
                nc.tensor.matmul(ops, lhsT=wo_sb[:, h, ts(ot, 128)],
                                 rhs=ctxn[h], start=(h == 0), stop=(h == H - 1))
            ob = work.tile([128, R], f32, tag="ob")
            nc.vector.tensor_copy(out=ob, in_=ops)
            nc.sync.dma_start(out=outT[ot], in_=ob)

    nc.compile()
    return nc


# ---------------------------------------------------------------- host side
def host_prep(cfg, x, wq, wk, wv, wo, pos):
    """x: [S, HID] f32, weights as in reference, pos: [S] int. Returns list of
    per-core input dicts."""
    R, W, HID, H, KVH, D, TH = (cfg["R"], cfg["W"], cfg["HID"], cfg["H"],
                                cfg["KVH"], cfg["D"], cfg["THETA"])
    KVW, HC, NKC = _derived(cfg)
    S = x.shape[0]
    ncores = S // R
    inv_freq = (1.0 / TH ** (np.arange(0, D, 2, dtype=np.float64) / D)).astype(
        np.float64)

    def pack_pm(wt, ncol):
        # [HID_or_CTX, ncol] weight.T -> [128, nchunk*ncol] partition-major
        a = wt.reshape(-1, 128, ncol)            # [chunks, 128, ncol]
        return np.ascontiguousarray(
            a.transpose(1, 0, 2).reshape(128, -1).astype(np.float16))

    wqT = pack_pm(wq.T, H * D)
    wkT = pack_pm(wk.T, KVH * D)
    wvT = pack_pm(wv.T, KVH * D)
    woT = pack_pm(wo.T, HID)

    in_maps = []
    for c in range(ncores):
        lo, hi = c * R - W, c * R + R
        pad = max(0, -lo)
        xw = np.zeros((KVW, HID), np.float32)
        xw[pad:] = x[max(lo, 0):hi]
        xTa = xw.T.reshape(HC, 128, KVW).astype(np.float16)   # [c, p, j]
        parts = []
        for b0 in range(0, KVW, 512):
            bw = min(512, KVW - b0)
            blk = xTa[:, :, b0:b0 + bw].transpose(1, 0, 2)     # [p, c, j]
            parts.append(np.ascontiguousarray(blk).reshape(-1))
        xT = np.concatenate(parts)

        pw = np.zeros(KVW, np.float64)
        pw[pad:] = pos[max(lo, 0):hi].astype(np.float64)
        ang = pw[:, None] * inv_freq[None, :]          # [KVW, 64]
        ck, sk = np.cos(ang).T, np.sin(ang).T          # [64, KVW]
        cosK32 = np.concatenate([ck, ck], 0).astype(np.float32)
        sinK32 = np.concatenate([-sk, sk], 0).astype(np.float32)
        scale = 1.0 / np.sqrt(D)
        cosQ = (cosK32[:, W:] * scale).astype(np.float16)
        sinQ = (sinK32[:, W:] * scale).astype(np.float16)
        cosK = cosK32.astype(np.float16)
        sinK = sinK32.astype(np.float16)
        i_idx = np.arange(R, dtype=np.float32)
        npad = np.maximum(0.0, pad - 1.0 - i_idx)[None, :].astype(np.float32)

        in_maps.append(dict(xT=xT, wqT=wqT, wkT=wkT, wvT=wvT, woT=woT,
                            cosK=cosK, sinK=sinK, cosQ=cosQ, sinQ=sinQ,
                            npadQ=npad))
    return in_maps


def assemble(cfg, outs):
    """outs: list of per-core outT arrays [HC, 128, R] -> [S, HID] f32."""
    R, HID = cfg["R"], cfg["HID"]
    blocks = [o.transpose(2, 0, 1).reshape(R, HID) for o in outs]
    return np.concatenate(blocks, 0).astype(np.float32)


_PROGRAM_CACHE = {}


def kernel(hidden_states, wq, wk, wv, wo, position_ids):
    from concourse.bass_utils import run_bass_kernel_spmd

    cfg = FULL_CFG
    x = np.asarray(hidden_states, np.float32)
    assert x.ndim == 3 and x.shape[0] == 1
    x2 = x[0]
    pos = np.asarray(position_ids)[0]
    in_maps = host_prep(cfg, x2, np.asarray(wq, np.float32),
                        np.asarray(wk, np.float32), np.asarray(wv, np.float32),
                        np.asarray(wo, np.float32), pos)
    key = "full"
    if key not in _PROGRAM_CACHE:
        _PROGRAM_CACHE[key] = build_program(cfg)
    nc = _PROGRAM_CACHE[key]
    res = run_bass_kernel_spmd(nc, in_maps, list(range(cfg["NCORES"])))
    outs = [res.results[i]["outT"] for i in range(cfg["NCORES"])]
    out = assemble(cfg, outs)
    return out.reshape(1, *out.shape)


# revision 18
# speedup vs baseline: 1.1651x; 1.1651x over previous
"""Trainium2 Bass kernel for Mistral sliding-window attention (B=1, S=4096,
HID=1024, H=8 q-heads, KVH=2 kv-heads, D=128, WINDOW=2048).

Strategy: shard the 4096-token sequence across 8 NeuronCores (512 queries per
core). Each core recomputes the K/V projections for its 2560-row key window
(own 512 rows + previous 2048), applies RoPE, computes sliding-window causal
attention for all 8 heads in the [keys, queries] orientation (scores^T), and
runs the full o_proj for its 512 rows. No collectives are needed; the host
concatenates the per-core 512-row output blocks.

Softmax is computed without max-subtraction (scores are O(1) here): P=exp(s)
via ScalarE with the denominator obtained as an extra ones-row matmul on the
TensorE, corrected on device for zero-padded (out-of-range) keys via a
host-provided count, since padded keys contribute exp(0)=1 to the sum but 0
to P@V (their V rows are 0).
"""

import sys
import numpy as np
from contextlib import ExitStack

if "/opt/trn_rl_repo" not in sys.path:
    sys.path.insert(0, "/opt/trn_rl_repo")

# ---------------------------------------------------------------- constants
FULL_CFG = dict(
    R=512,        # queries per core
    W=2048,       # sliding window
    HID=1024,     # hidden size
    H=8,          # query heads
    KVH=2,        # kv heads
    D=128,        # head dim
    THETA=10000.0,
    GRP=2,        # score chunks per exp batch
    NCORES=8,
)


def _derived(cfg):
    R, W, HID = cfg["R"], cfg["W"], cfg["HID"]
    KVW = W + R
    HC = HID // 128
    NKC = KVW // 128
    assert W >= R and R <= 512 and HID % 128 == 0 and KVW % 128 == 0
    return KVW, HC, NKC


def build_program(cfg):
    import concourse.bass as bass
    import concourse.tile as tile
    from concourse import bacc, mybir

    f32, f16 = mybir.dt.float32, mybir.dt.float16
    ts = bass.ts
    R, W, HID, H, KVH, D, GRP = (cfg["R"], cfg["W"], cfg["HID"], cfg["H"],
                                 cfg["KVH"], cfg["D"], cfg["GRP"])
    KVW, HC, NKC = _derived(cfg)
    GROUPS = H // KVH
    LO = R // 128          # lower-edge chunks: kc in [0, LO)
    HIE = W // 128         # upper-edge chunks: kc in [HIE, NKC)

    nc = bacc.Bacc("TRN2", target_bir_lowering=False, debug=False)

    def din(name, shape, dt):
        return nc.dram_tensor(name, shape, dt, kind="ExternalInput").ap()

    xT = din("xT", [128 * HC * KVW], f16)       # block-major packed
    wqT = din("wqT", [128, HC * H * D], f16)    # partition-major packed
    wkT = din("wkT", [128, HC * KVH * D], f16)
    wvT = din("wvT", [128, HC * KVH * D], f16)
    woT = din("woT", [128, H * HID], f16)
    cosK = din("cosK", [128, KVW], f16)
    sinK = din("sinK", [128, KVW], f16)
    cosQ = din("cosQ", [128, R], f16)
    sinQ = din("sinQ", [128, R], f16)
    npadQ = din("npadQ", [1, R], f32)
    outT = nc.dram_tensor("outT", [HC, 128, R], f32, kind="ExternalOutput").ap()

    with tile.TileContext(nc) as tc, ExitStack() as ctx:
        const = ctx.enter_context(tc.tile_pool(name="const", bufs=1))
        kvp = ctx.enter_context(tc.tile_pool(name="kvp", bufs=1))
        work = ctx.enter_context(tc.tile_pool(name="work", bufs=2))
        dramp = ctx.enter_context(tc.tile_pool(name="dramp", bufs=2, space="DRAM"))
        psG = ctx.enter_context(tc.tile_pool(name="psG", bufs=2, space="PSUM"))
        psM = ctx.enter_context(tc.tile_pool(name="psM", bufs=1, space="PSUM"))
        psC = ctx.enter_context(tc.tile_pool(name="psC", bufs=2, space="PSUM"))
        psD = ctx.enter_context(tc.tile_pool(name="psD", bufs=1, space="PSUM"))

        # ---------------- input loads
        wk_sb = const.tile([128, HC, KVH * D], f16)
        wv_sb = const.tile([128, HC, KVH * D], f16)
        nc.sync.dma_start(out=wk_sb, in_=wkT)
        nc.sync.dma_start(out=wv_sb, in_=wvT)
        cosK_sb = const.tile([128, KVW], f16)
        sinK_sb = const.tile([128, KVW], f16)
        cosQ_sb = const.tile([128, R], f16)
        sinQ_sb = const.tile([128, R], f16)
        npad_sb = const.tile([1, R], f32)
        xT_sb = const.tile([128, HC, KVW], f16)
        xblocks = []
        xoff = 0
        for b0, bw in ((o, min(512, KVW - o)) for o in range(0, KVW, 512)):
            xblocks.append((b0, bw, xoff))
            xoff += 128 * HC * bw

        def emit_xt_block(i):
            b0, bw, off = xblocks[i]
            src_ap = bass.AP(tensor=xT.tensor, offset=off,
                             ap=[[HC * bw, 128], [bw, HC], [1, bw]])
            nc.sync.dma_start(out=xT_sb[:, :, b0:b0 + bw], in_=src_ap)

        for dst, src in ((cosK_sb, cosK), (sinK_sb, sinK), (cosQ_sb, cosQ),
                         (sinQ_sb, sinQ), (npad_sb, npadQ)):
            nc.sync.dma_start(out=dst, in_=src)
        for i in range(len(xblocks)):
            emit_xt_block(i)
        wq_sb = const.tile([128, HC, H * D], f16)
        wo_sb = const.tile([128, H, HID], f16)

        def emit_late_loads():
            gate = gate_box[0]
            di = nc.sync.dma_start(out=wq_sb, in_=wqT)
            tile.add_dep_helper(di.ins, gate.ins, sync=True,
                                reason="delay wq load")
            di = nc.sync.dma_start(out=wo_sb, in_=woT)
            tile.add_dep_helper(di.ins, gate.ins, sync=True,
                                reason="delay wo load")
        ones_sb = const.tile([128, 1], f16)
        nc.vector.memset(ones_sb, 1.0)
        ones_row = const.tile([1, 128], f32)
        nc.vector.memset(ones_row, 1.0)

        # ---------------- additive masks for edge chunks (compile-time)
        # scores^T chunk kc holds keys jl = 128*kc + kp (kp = partition) vs
        # queries i (free). valid iff i < jl <= i + W.
        masks = {}
        for kc in list(range(LO)) + list(range(HIE, NKC)):
            m = const.tile([128, R], f16, name=f"mask{kc}")
            nc.gpsimd.memset(m, 1.0)
            if kc < LO:
                # valid iff 128*kc + kp - 1 - i >= 0
                nc.gpsimd.affine_select(
                    out=m, in_=m, compare_op=mybir.AluOpType.is_ge, fill=0.0,
                    base=128 * kc - 1, pattern=[[-1, R]], channel_multiplier=1)
            else:
                # valid iff i - kp + (W - 128*kc) >= 0
                nc.gpsimd.affine_select(
                    out=m, in_=m, compare_op=mybir.AluOpType.is_ge, fill=0.0,
                    base=W - 128 * kc, pattern=[[1, R]], channel_multiplier=-1)
            masks[kc] = m

        # ---------------- RoPE helper (operates on [128, width] psum -> sbuf)
        def rope(dst, src_ps, cos_ap, sin_ap, width):
            sb = work.tile([128, width], f32, tag="ropesrc")
            nc.vector.tensor_copy(out=sb, in_=src_ps)
            tmp = work.tile([128, width], f32, tag="rtmp")
            nc.vector.tensor_copy(out=tmp[0:64, :], in_=sb[64:128, :])
            nc.vector.tensor_copy(out=tmp[64:128, :], in_=sb[0:64, :])
            ta = work.tile([128, width], f32, tag="ra")
            nc.vector.tensor_mul(ta, sb, cos_ap)
            tb = work.tile([128, width], f32, tag="rb2")
            nc.vector.tensor_mul(tb, tmp, sin_ap)
            return nc.vector.tensor_add(dst, ta, tb)

        def blocks(total, step):
            out = []
            o = 0
            while o < total:
                out.append((o, min(step, total - o)))
                o += step
            return out

        # ---------------- K/V projections + RoPE (per kv head)
        kT_sb = [kvp.tile([128, KVW], f16, name=f"kT{g}") for g in range(KVH)]
        v_sb = [kvp.tile([128, NKC, 128], f16, name=f"v{g}") for g in range(KVH)]

        gate_box = []

        def emit_kv(g):
            for b0, bw in blocks(KVW, 512):
                kps = psM.tile([128, 512], f32, tag="mm")
                for c in range(HC):
                    nc.tensor.matmul(kps[:, :bw],
                                     lhsT=wk_sb[:, c, g * D:(g + 1) * D],
                                     rhs=xT_sb[:, c, b0:b0 + bw],
                                     start=(c == 0), stop=(c == HC - 1))
                rinst = rope(kT_sb[g][:, b0:b0 + bw], kps[:, :bw],
                             cosK_sb[:, b0:b0 + bw], sinK_sb[:, b0:b0 + bw], bw)
                if not gate_box:
                    gate_box.append(rinst)
                vps = psD.tile([128, 512], f32, tag="den")
                for c in range(HC):
                    nc.tensor.matmul(vps[:, :bw],
                                     lhsT=wv_sb[:, c, g * D:(g + 1) * D],
                                     rhs=xT_sb[:, c, b0:b0 + bw],
                                     start=(c == 0), stop=(c == HC - 1))
                vT_t = work.tile([128, 512], f16, tag="vT")
                nc.vector.tensor_copy(out=vT_t[:, :bw], in_=vps[:, :bw])
                c0 = b0 // 128
                nc.sync.dma_start_transpose(
                    out=v_sb[g][:, c0:c0 + bw // 128, :], in_=vT_t[:, :bw])

        # ---------------- attention per query head
        ctxn = [kvp.tile([128, R], f16, name=f"ctxn{h}") for h in range(H)]
        chunk_groups = [list(range(s, min(s + GRP, NKC)))
                        for s in range(0, NKC, GRP)]

        qTs = {}

        def emit_q(h):
            qps = psM.tile([128, 512], f32, tag="mm")
            for c in range(HC):
                nc.tensor.matmul(qps[:, :R], lhsT=wq_sb[:, c, h * D:(h + 1) * D],
                                 rhs=xT_sb[:, c, W:KVW],
                                 start=(c == 0), stop=(c == HC - 1))
            qT = work.tile([128, R], f16, tag="qT")
            rope(qT, qps[:, :R], cosQ_sb, sinQ_sb, R)
            qTs[h] = qT

        def emit_attn(h):
            g = h // GROUPS
            qT = qTs.pop(h)

            ctx_ps = psC.tile([128, R], f32, tag="ctx")
            den_ps = psD.tile([1, R], f32, tag="den")
            for gi, grp in enumerate(chunk_groups):
                if gi == len(chunk_groups) - 1 and h + 1 < H:
                    emit_q(h + 1)
                gw = len(grp) * R
                scps = psG.tile([128, GRP * R], f32, tag="sc")
                for j, kc in enumerate(grp):
                    nc.tensor.matmul(scps[:, ts(j, R)],
                                     lhsT=kT_sb[g][:, ts(kc, 128)], rhs=qT,
                                     start=True, stop=True)
                P_sb = work.tile([128, GRP * R], f16, tag="P", bufs=3)
                nc.scalar.activation(out=P_sb[:, :gw], in_=scps[:, :gw],
                                     func=mybir.ActivationFunctionType.Exp)
                for j, kc in enumerate(grp):
                    if kc in masks:
                        nc.vector.tensor_mul(P_sb[:, ts(j, R)],
                                             P_sb[:, ts(j, R)], masks[kc])
                for j, kc in enumerate(grp):
                    nc.tensor.matmul(ctx_ps, lhsT=v_sb[g][:, kc, :],
                                     rhs=P_sb[:, ts(j, R)],
                                     start=(kc == 0), stop=(kc == NKC - 1))
                    nc.tensor.matmul(den_ps, lhsT=ones_sb,
                                     rhs=P_sb[:, ts(j, R)],
                                     start=(kc == 0), stop=(kc == NKC - 1))

            drow = work.tile([1, R], f32, tag="drow")
            nc.vector.tensor_sub(drow, den_ps, npad_sb)
            rrow = work.tile([1, R], f32, tag="rrow")
            nc.vector.reciprocal_approx_fast(out=rrow, in_=drow)
            rbc = work.tile([128, R], f32, tag="rbc")
            if h == H - 1:
                bc_ps = psM.tile([128, 512], f32, tag="mm")
                nc.tensor.matmul(bc_ps[:, :R], lhsT=ones_row, rhs=rrow,
                                 start=True, stop=True)
                nc.vector.tensor_copy(out=rbc, in_=bc_ps[:, :R])
            else:
                rdram = dramp.tile([1, R], f32, tag="rdram")
                nc.sync.dma_start(out=rdram, in_=rrow)
                rbc_src = bass.AP(tensor=rdram.tensor, offset=rdram.offset,
                                  ap=[[0, 128]] + list(rdram.ap[-1:]))
                nc.sync.dma_start(out=rbc, in_=rbc_src)
            nc.vector.tensor_mul(ctxn[h], ctx_ps, rbc)

        emit_kv(0)
        emit_late_loads()
        for g in range(1, KVH):
            emit_kv(g)
        emit_q(0)
        for h in range(H):
            emit_attn(h)

        # ---------------- o_proj
        for ot in range(HC):
            ops = psC.tile([128, R], f32, tag="ctx")
            for h in range(H):
                nc.tensor.matmul(ops, lhsT=wo_sb[:, h, ts(ot, 128)],
                                 rhs=ctxn[h], start=(h == 0), stop=(h == H - 1))
            ob = work.tile([128, R], f32, tag="ob")
            nc.vector.tensor_copy(out=ob, in_=ops)
            nc.sync.dma_start(out=outT[ot], in_=ob)

    nc.compile()
    return nc


# ---------------------------------------------------------------- host side
def host_prep(cfg, x, wq, wk, wv, wo, pos):
    """x: [S, HID] f32, weights as in reference, pos: [S] int. Returns list of
    per-core input dicts."""
    R, W, HID, H, KVH, D, TH = (cfg["R"], cfg["W"], cfg["HID"], cfg["H"],
                                cfg["KVH"], cfg["D"], cfg["THETA"])
    KVW, HC, NKC = _derived(cfg)
    S = x.shape[0]
    ncores = S // R
    inv_freq = (1.0 / TH ** (np.arange(0, D, 2, dtype=np.float64) / D)).astype(
        np.float64)

    def pack_pm(wt, ncol):
        # [HID_or_CTX, ncol] weight.T -> [128, nchunk*ncol] partition-major
        a = wt.reshape(-1, 128, ncol)            # [chunks, 128, ncol]
        return np.ascontiguousarray(
            a.transpose(1, 0, 2).reshape(128, -1).astype(np.float16))

    wqT = pack_pm(wq.T, H * D)
    wkT = pack_pm(wk.T, KVH * D)
    wvT = pack_pm(wv.T, KVH * D)
    woT = pack_pm(wo.T, HID)

    in_maps = []
    for c in range(ncores):
        lo, hi = c * R - W, c * R + R
        pad = max(0, -lo)
        xw = np.zeros((KVW, HID), np.float32)
        xw[pad:] = x[max(lo, 0):hi]
        xTa = xw.T.reshape(HC, 128, KVW).astype(np.float16)   # [c, p, j]
        parts = []
        for b0 in range(0, KVW, 512):
            bw = min(512, KVW - b0)
            blk = xTa[:, :, b0:b0 + bw].transpose(1, 0, 2)     # [p, c, j]
            parts.append(np.ascontiguousarray(blk).reshape(-1))
        xT = np.concatenate(parts)

        pw = np.zeros(KVW, np.float64)
        pw[pad:] = pos[max(lo, 0):hi].astype(np.float64)
        ang = pw[:, None] * inv_freq[None, :]          # [KVW, 64]
        ck, sk = np.cos(ang).T, np.sin(ang).T          # [64, KVW]
        cosK32 = np.concatenate([ck, ck], 0).astype(np.float32)
        sinK32 = np.concatenate([-sk, sk], 0).astype(np.float32)
        scale = 1.0 / np.sqrt(D)
        cosQ = (cosK32[:, W:] * scale).astype(np.float16)
        sinQ = (sinK32[:, W:] * scale).astype(np.float16)
        cosK = cosK32.astype(np.float16)
        sinK = sinK32.astype(np.float16)
        i_idx = np.arange(R, dtype=np.float32)
        npad = np.maximum(0.0, pad - 1.0 - i_idx)[None, :].astype(np.float32)

        in_maps.append(dict(xT=xT, wqT=wqT, wkT=wkT, wvT=wvT, woT=woT,
                            cosK=cosK, sinK=sinK, cosQ=cosQ, sinQ=sinQ,
                            npadQ=npad))
    return in_maps


def assemble(cfg, outs):
    """outs: list of per-core outT arrays [HC, 128, R] -> [S, HID] f32."""
    R, HID = cfg["R"], cfg["HID"]
    blocks = [o.transpose(2, 0, 1).reshape(R, HID) for o in outs]
    return np.concatenate(blocks, 0).astype(np.float32)


_PROGRAM_CACHE = {}


def kernel(hidden_states, wq, wk, wv, wo, position_ids):
    from concourse.bass_utils import run_bass_kernel_spmd

    cfg = FULL_CFG
    x = np.asarray(hidden_states, np.float32)
    assert x.ndim == 3 and x.shape[0] == 1
    x2 = x[0]
    pos = np.asarray(position_ids)[0]
    in_maps = host_prep(cfg, x2, np.asarray(wq, np.float32),
                        np.asarray(wk, np.float32), np.asarray(wv, np.float32),
                        np.asarray(wo, np.float32), pos)
    key = "full"
    if key not in _PROGRAM_CACHE:
        _PROGRAM_CACHE[key] = build_program(cfg)
    nc = _PROGRAM_CACHE[key]
    res = run_bass_kernel_spmd(nc, in_maps, list(range(cfg["NCORES"])))
    outs = [res.results[i]["outT"] for i in range(cfg["NCORES"])]
    out = assemble(cfg, outs)
    return out.reshape(1, *out.shape)
